# revision 11
# baseline (speedup 1.0000x reference)
"""Multi-head causal attention (B=4, T=2048, DM=1024, H=16, dk=dv=64) on 8
Trainium2 NeuronCores.

Sharding: core c handles batch b = c//2 and head-group g = c%2 (8 heads).
Data-parallel over batch x tensor-parallel over heads; no cross-core comm.

Per-core bass/Tile kernel (all matmuls bf16, PSUM accumulation fp32):
  - host pre-lays-out x^T (d on partitions), Wq||Wk stacked per head, Wv
    packed across heads, and the causal mask tiles, all in bf16.
  - projections: qT/kT = (Wq||Wk)^T-stationary matmuls vs x^T;
    v in natural [t, dv] layout via x^T-stationary matmuls vs packed Wv.
  - attention, flash-style over 512-wide t-chunks and 128-wide s-tiles:
      S^T[s,t] = kT_slice.T @ qT_chunk          (PE, K=64)
      P = exp(S * dk^-0.5)                       (ScalarE, scale folded in)
      diagonal tiles: P *= causal 0/1 mask       (VectorE)
      O_aug^T[65, t] += [v | 1]^T-stationary @ P (PE, K=128, fp32 accum)
    row 64 of O_aug^T collects the softmax denominators.
  - O_aug^T chunks are copied to SBUF and DMAed out unnormalized;
    the host does the final divide + transpose (O(T*DV) work).
"""
import numpy as np
import ml_dtypes

_BF16 = ml_dtypes.bfloat16

B, T, DM = 4, 2048, 1024
H, DK, DV = 16, 64, 64
N_CORES = 8
HPC = 8          # heads per core
NDC = DM // 128  # 8 d-chunks
NTT = T // 128   # 16 t/s tiles of 128
NTC = T // 512   # 4 t-chunks of 512

_cached = None   # (nc, run_bass_kernel_spmd)

# Set by a driver (e.g. test.py) to collect an NTFF profile; the exec time
# lands in LAST_EXEC_NS.
TRACE = False
LAST_EXEC_NS = None


def _build_program():
    global _cached
    if _cached is not None:
        return _cached
    import concourse.bacc as bacc
    import concourse.mybir as mybir
    from concourse import tile

    bf16 = mybir.dt.bfloat16
    f32 = mybir.dt.float32
    Exp = mybir.ActivationFunctionType.Exp

    nc = bacc.Bacc()
    xt = nc.declare_dram_parameter("xt", [128, NDC, T], bf16, isOutput=False)
    wqk = nc.declare_dram_parameter("wqk", [128, HPC, NDC, 128], bf16, isOutput=False)
    wv = nc.declare_dram_parameter("wv", [128, NDC, 512], bf16, isOutput=False)
    msk = nc.declare_dram_parameter("msk", [128, 2048], bf16, isOutput=False)
    ot = nc.declare_dram_parameter("ot", [HPC, DV + 1, T], f32, isOutput=True)

    with tile.TileContext(nc) as tc:
        with (
            tc.tile_pool(name="consts", bufs=1) as consts,
            tc.tile_pool(name="vpool", bufs=1) as vpool,
            tc.tile_pool(name="qk", bufs=4) as qkpool,
            tc.tile_pool(name="pt", bufs=4) as ptpool,
            tc.tile_pool(name="osb", bufs=2) as opool,
            tc.tile_pool(name="proj_ps", bufs=1, space="PSUM") as proj_ps,
            tc.tile_pool(name="s_ps", bufs=3, space="PSUM") as s_ps,
            tc.tile_pool(name="o_ps", bufs=1, space="PSUM") as o_ps,
        ):
            # Loads are t-chunked and ordered so the first MB unblocks the
            # first projection tiles while the rest streams in.
            wv_sb = consts.tile([128, NDC, 512], bf16)
            msk_sb = consts.tile([128, 2048], bf16)
            xt_sb = [
                consts.tile([128, T], bf16, name=f"xt{dc}", tag=f"xt{dc}")
                for dc in range(NDC)
            ]
            wqk_sb = consts.tile([128, HPC, NDC, 128], bf16)
            for dc in range(NDC):
                nc.sync.dma_start(xt_sb[dc][:, 0:512], xt[:, dc, 0:512])
            nc.sync.dma_start(wv_sb[:], wv[:])
            nc.sync.dma_start(wqk_sb[:, 0], wqk[:, 0])
            for tch in range(1, NTC):
                sl = slice(512 * tch, 512 * (tch + 1))
                for dc in range(NDC):
                    nc.sync.dma_start(xt_sb[dc][:, sl], xt[:, dc, sl])
            nc.sync.dma_start(msk_sb[:], msk[:])
            for h in range(1, HPC):
                nc.sync.dma_start(wqk_sb[:, h], wqk[:, h])

            # V projection, all heads at once: v_sb[s, j, h, 0:64] = v values,
            # v_sb[s, j, h, 64] = 1.0 (softmax-denominator column).
            v_sb = vpool.tile([128, NTT, HPC, DV + 1], bf16)
            nc.gpsimd.memset(v_sb[:, :, :, DV], 1.0)
            for tt in range(NTT):
                ps = proj_ps.tile([128, 512], f32, name="ps_v", tag="ps_qk")
                for dc in range(NDC):
                    nc.tensor.matmul(
                        ps[:],
                        lhsT=xt_sb[dc][:, 128 * tt:128 * (tt + 1)],
                        rhs=wv_sb[:, dc, :],
                        start=(dc == 0),
                        stop=(dc == NDC - 1),
                    )
                nc.vector.tensor_copy(
                    v_sb[:, tt, :, 0:DV],
                    ps[:].rearrange("p (h e) -> p h e", h=HPC),
                )

            def qk_proj(h):
                # QK projection for head h: psum rows 0:64 hold q^T chunk,
                # rows 64:128 hold k^T chunk.
                # qk1 = [q | k] on partitions [0:64 | 64:128];
                # qk2 = [k | q] (swapped halves).  Row-packed S matmuls need
                # weights and fmap at the SAME base partition, so even s-tiles
                # use (k,q) from partitions 0:64 and odd s-tiles use (k,q)
                # from partitions 64:128.
                qk1 = qkpool.tile([128, T], bf16, name=f"qk1_{h}", tag="qk1")
                qk2 = qkpool.tile([128, T], bf16, name=f"qk2_{h}", tag="qk2")
                for tch in range(NTC):
                    ps = proj_ps.tile([128, 512], f32, name="ps_qk", tag="ps_qk")
                    for dc in range(NDC):
                        nc.tensor.matmul(
                            ps[:],
                            lhsT=wqk_sb[:, h, dc, :],
                            rhs=xt_sb[dc][:, 512 * tch:512 * (tch + 1)],
                            start=(dc == 0),
                            stop=(dc == NDC - 1),
                        )
                    sl = slice(512 * tch, 512 * (tch + 1))
                    nc.vector.tensor_copy(qk1[:, sl], ps[:])
                    # swapped halves, cheap SBUF->SBUF bf16 copies
                    nc.vector.tensor_copy(qk2[0:64, sl], qk1[64:128, sl])
                    nc.vector.tensor_copy(qk2[64:128, sl], qk1[0:64, sl])
                return qk1, qk2

            def attn_chunk(h, qk1, qk2, c):
                # Attention for head h, one 512-wide t-chunk, causal.
                po = o_ps.tile([DV + 1, 512], f32, name="po", tag="po")
                jmax = 4 * c + 3        # last s-tile index for this chunk
                csl = slice(512 * c, 512 * (c + 1))
                for pair in range(2 * (c + 1)):
                    pS = s_ps.tile([128, 1024], f32, name="pS", tag="pS")
                    j0 = 2 * pair
                    # two concurrent K=64 matmuls in disjoint PE row groups
                    nc.tensor.matmul(
                        pS[:, 0:512],
                        lhsT=qk2[0:64, 128 * j0:128 * (j0 + 1)],
                        rhs=qk1[0:64, csl],
                        start=True,
                        stop=True,
                        tile_position=(0, 0),
                    )
                    nc.tensor.matmul(
                        pS[:, 512:1024],
                        lhsT=qk1[64:128, 128 * (j0 + 1):128 * (j0 + 2)],
                        rhs=qk2[64:128, csl],
                        start=True,
                        stop=True,
                        tile_position=(64, 0),
                    )
                    pt = ptpool.tile([128, 1024], bf16, name="pt", tag="pt")
                    nc.scalar.activation(pt[:], pS[:], Exp, scale=DK ** -0.5)
                    if 2 * pair >= 4 * c:
                        # pair overlaps the diagonal: r = relative s-tile
                        # index (0 or 2) within the chunk
                        r = 2 * pair - 4 * c
                        nc.vector.tensor_mul(
                            pt[:], pt[:], msk_sb[:, 512 * r:512 * r + 1024]
                        )
                    for u in (0, 1):
                        j = 2 * pair + u
                        nc.tensor.matmul(
                            po[:],
                            lhsT=v_sb[:, j, h, :],
                            rhs=pt[:, 512 * u:512 * (u + 1)],
                            start=(j == 0),
                            stop=(j == jmax),
                        )
                o_sb = opool.tile([DV + 1, 512], f32, name="o_sb", tag="o_sb")
                nc.vector.tensor_copy(o_sb[:], po[:])
                nc.sync.dma_start(ot[h, :, csl], o_sb[:])

            # Process heads in pairs with their attention chunks interleaved:
            # while head A waits on exp (ScalarE), PE runs head B's matmuls.
            for hp in range(HPC // 2):
                hA, hB = 2 * hp, 2 * hp + 1
                qkA = qk_proj(hA)
                qkB = qk_proj(hB)
                for c in range(NTC):
                    attn_chunk(hA, *qkA, c)
                    attn_chunk(hB, *qkB, c)

    nc.finalize()
    from concourse.bass_utils import run_bass_kernel_spmd
    _cached = (nc, run_bass_kernel_spmd)
    return _cached


def _prep_core_inputs(x, Wq, Wk, Wv, core):
    b, g = core // 2, core % 2
    xb = x[b].astype(_BF16)                                  # [T, DM]
    xt = np.ascontiguousarray(
        xb.T.reshape(NDC, 128, T).transpose(1, 0, 2)         # [p, dc, t]
    )
    wq = Wq[HPC * g:HPC * (g + 1)].astype(_BF16)             # [8, DM, 64]
    wk = Wk[HPC * g:HPC * (g + 1)].astype(_BF16)
    wv = Wv[HPC * g:HPC * (g + 1)].astype(_BF16)
    wqk = np.concatenate([wq, wk], axis=2)                   # [h, DM, 128]
    wqk = np.ascontiguousarray(
        wqk.reshape(HPC, NDC, 128, 128).transpose(2, 0, 1, 3)  # [p, h, dc, f]
    )
    wvp = np.ascontiguousarray(
        wv.reshape(HPC, NDC, 128, DV).transpose(2, 1, 0, 3).reshape(128, NDC, 512)
    )
    return {"xt": xt, "wqk": wqk, "wv": wvp, "msk": _mask()}


_mask_cache = None


def _mask():
    global _mask_cache
    if _mask_cache is None:
        p = np.arange(128)[:, None, None]
        r = np.arange(4)[None, :, None]
        t = np.arange(512)[None, None, :]
        _mask_cache = (128 * r + p <= t).astype(_BF16).reshape(128, 2048)
    return _mask_cache


def kernel(x, Wq, Wk, Wv):
    global LAST_EXEC_NS
    nc, run_spmd = _build_program()
    in_maps = [_prep_core_inputs(x, Wq, Wk, Wv, c) for c in range(N_CORES)]
    res = run_spmd(nc, in_maps, list(range(N_CORES)), trace=TRACE)
    global _LAST_RES
    _LAST_RES = res
    LAST_EXEC_NS = res.exec_time_ns

    out = np.empty((B, T, H * DV), np.float32)
    for c in range(N_CORES):
        b, g = c // 2, c % 2
        otc = res.results[c]["ot"]                 # [8, 65, T]
        o = otc[:, :DV, :] / otc[:, DV:DV + 1, :]  # [h, dv, t]
        out[b, :, 512 * g:512 * (g + 1)] = (
            o.transpose(2, 0, 1).reshape(T, HPC * DV)
        )
    return out


# revision 12
# speedup vs baseline: 1.0631x; 1.0631x over previous
"""Multi-head causal attention (B=4, T=2048, DM=1024, H=16, dk=dv=64) on 8
Trainium2 NeuronCores.

Sharding: core c handles batch b = c//2 and head-group g = c%2 (8 heads).
Data-parallel over batch x tensor-parallel over heads; no cross-core comm.

Per-core bass/Tile kernel (all matmuls bf16, PSUM accumulation fp32):
  - host pre-lays-out x^T (d on partitions), Wq||Wk stacked per head, Wv
    packed across heads, and the causal mask tiles, all in bf16.
  - projections: qT/kT = (Wq||Wk)^T-stationary matmuls vs x^T;
    v in natural [t, dv] layout via x^T-stationary matmuls vs packed Wv.
  - attention, flash-style over 512-wide t-chunks and 128-wide s-tiles:
      S^T[s,t] = kT_slice.T @ qT_chunk          (PE, K=64)
      P = exp(S * dk^-0.5)                       (ScalarE, scale folded in)
      diagonal tiles: P *= causal 0/1 mask       (VectorE)
      O_aug^T[65, t] += [v | 1]^T-stationary @ P (PE, K=128, fp32 accum)
    row 64 of O_aug^T collects the softmax denominators.
  - O_aug^T chunks are copied to SBUF and DMAed out unnormalized;
    the host does the final divide + transpose (O(T*DV) work).
"""
import numpy as np
import ml_dtypes

_BF16 = ml_dtypes.bfloat16

B, T, DM = 4, 2048, 1024
H, DK, DV = 16, 64, 64
N_CORES = 8
HPC = 8          # heads per core
NDC = DM // 128  # 8 d-chunks
NTT = T // 128   # 16 t/s tiles of 128
NTC = T // 512   # 4 t-chunks of 512

_cached = None   # (nc, run_bass_kernel_spmd)

# Set by a driver (e.g. test.py) to collect an NTFF profile; the exec time
# lands in LAST_EXEC_NS.
TRACE = False
LAST_EXEC_NS = None


def _build_program():
    global _cached
    if _cached is not None:
        return _cached
    import concourse.bacc as bacc
    import concourse.mybir as mybir
    from concourse import tile

    bf16 = mybir.dt.bfloat16
    f32 = mybir.dt.float32
    Exp = mybir.ActivationFunctionType.Exp

    nc = bacc.Bacc()
    xt = nc.declare_dram_parameter("xt", [128, NDC, T], bf16, isOutput=False)
    wqk = nc.declare_dram_parameter("wqk", [128, HPC, NDC, 128], bf16, isOutput=False)
    wv = nc.declare_dram_parameter("wv", [128, NDC, 512], bf16, isOutput=False)
    msk = nc.declare_dram_parameter("msk", [128, 2048], bf16, isOutput=False)
    ot = nc.declare_dram_parameter("ot", [HPC, DV + 1, T], f32, isOutput=True)

    with tile.TileContext(nc) as tc:
        with (
            tc.tile_pool(name="consts", bufs=1) as consts,
            tc.tile_pool(name="vpool", bufs=1) as vpool,
            tc.tile_pool(name="qk", bufs=4) as qkpool,
            tc.tile_pool(name="pt", bufs=4) as ptpool,
            tc.tile_pool(name="osb", bufs=2) as opool,
            tc.tile_pool(name="proj_ps", bufs=2, space="PSUM") as proj_ps,
            tc.tile_pool(name="s_ps", bufs=2, space="PSUM") as s_ps,
            tc.tile_pool(name="o_ps", bufs=2, space="PSUM") as o_ps,
        ):
            # Loads are t-chunked and ordered so the first MB unblocks the
            # first projection tiles while the rest streams in.
            wv_sb = consts.tile([128, NDC, 512], bf16)
            msk_sb = consts.tile([128, 2048], bf16)
            xt_sb = [
                consts.tile([128, T], bf16, name=f"xt{dc}", tag=f"xt{dc}")
                for dc in range(NDC)
            ]
            wqk_sb = consts.tile([128, HPC, NDC, 128], bf16)
            for dc in range(NDC):
                nc.sync.dma_start(xt_sb[dc][:, 0:512], xt[:, dc, 0:512])
            nc.sync.dma_start(wv_sb[:], wv[:])
            nc.sync.dma_start(wqk_sb[:, 0], wqk[:, 0])
            for tch in range(1, NTC):
                sl = slice(512 * tch, 512 * (tch + 1))
                for dc in range(NDC):
                    nc.sync.dma_start(xt_sb[dc][:, sl], xt[:, dc, sl])
            nc.sync.dma_start(msk_sb[:], msk[:])
            for h in range(1, HPC):
                nc.sync.dma_start(wqk_sb[:, h], wqk[:, h])

            # V projection, all heads at once: v_sb[s, j, h, 0:64] = v values,
            # v_sb[s, j, h, 64] = 1.0 (softmax-denominator column).
            v_sb = vpool.tile([128, NTT, HPC, DV + 1], bf16)
            nc.gpsimd.memset(v_sb[:, :, :, DV], 1.0)
            for tt in range(NTT):
                ps = proj_ps.tile([128, 512], f32, name="ps_v", tag="ps_qk")
                for dc in range(NDC):
                    nc.tensor.matmul(
                        ps[:],
                        lhsT=xt_sb[dc][:, 128 * tt:128 * (tt + 1)],
                        rhs=wv_sb[:, dc, :],
                        start=(dc == 0),
                        stop=(dc == NDC - 1),
                    )
                nc.vector.tensor_copy(
                    v_sb[:, tt, :, 0:DV],
                    ps[:].rearrange("p (h e) -> p h e", h=HPC),
                )

            def qk_proj(h):
                # QK projection for head h: psum rows 0:64 hold q^T chunk,
                # rows 64:128 hold k^T chunk.
                # qk1 = [q | k] on partitions [0:64 | 64:128];
                # qk2 = [k | q] (swapped halves).  Row-packed S matmuls need
                # weights and fmap at the SAME base partition, so even s-tiles
                # use (k,q) from partitions 0:64 and odd s-tiles use (k,q)
                # from partitions 64:128.
                qk1 = qkpool.tile([128, T], bf16, name=f"qk1_{h}", tag="qk1")
                qk2 = qkpool.tile([128, T], bf16, name=f"qk2_{h}", tag="qk2")
                for tch in range(NTC):
                    ps = proj_ps.tile([128, 512], f32, name="ps_qk", tag="ps_qk")
                    for dc in range(NDC):
                        nc.tensor.matmul(
                            ps[:],
                            lhsT=wqk_sb[:, h, dc, :],
                            rhs=xt_sb[dc][:, 512 * tch:512 * (tch + 1)],
                            start=(dc == 0),
                            stop=(dc == NDC - 1),
                        )
                    sl = slice(512 * tch, 512 * (tch + 1))
                    nc.vector.tensor_copy(qk1[:, sl], ps[:])
                    # swapped halves, cheap SBUF->SBUF bf16 copies
                    nc.vector.tensor_copy(qk2[0:64, sl], qk1[64:128, sl])
                    nc.vector.tensor_copy(qk2[64:128, sl], qk1[0:64, sl])
                return qk1, qk2

            def attn_chunk(h, qk1, qk2, c):
                # Attention for head h, one 512-wide t-chunk, causal.
                po = o_ps.tile([DV + 1, 512], f32, name="po", tag="po")
                jmax = 4 * c + 3        # last s-tile index for this chunk
                csl = slice(512 * c, 512 * (c + 1))
                for pair in range(2 * (c + 1)):
                    pS = s_ps.tile([128, 1024], f32, name="pS", tag="pS")
                    j0 = 2 * pair
                    # two concurrent K=64 matmuls in disjoint PE row groups
                    nc.tensor.matmul(
                        pS[:, 0:512],
                        lhsT=qk2[0:64, 128 * j0:128 * (j0 + 1)],
                        rhs=qk1[0:64, csl],
                        start=True,
                        stop=True,
                        tile_position=(0, 0),
                    )
                    nc.tensor.matmul(
                        pS[:, 512:1024],
                        lhsT=qk1[64:128, 128 * (j0 + 1):128 * (j0 + 2)],
                        rhs=qk2[64:128, csl],
                        start=True,
                        stop=True,
                        tile_position=(64, 0),
                    )
                    pt = ptpool.tile([128, 1024], bf16, name="pt", tag="pt")
                    nc.scalar.activation(pt[:], pS[:], Exp, scale=DK ** -0.5)
                    if 2 * pair >= 4 * c:
                        # pair overlaps the diagonal: r = relative s-tile
                        # index (0 or 2) within the chunk
                        r = 2 * pair - 4 * c
                        nc.vector.tensor_mul(
                            pt[:], pt[:], msk_sb[:, 512 * r:512 * r + 1024]
                        )
                    for u in (0, 1):
                        j = 2 * pair + u
                        nc.tensor.matmul(
                            po[:],
                            lhsT=v_sb[:, j, h, :],
                            rhs=pt[:, 512 * u:512 * (u + 1)],
                            start=(j == 0),
                            stop=(j == jmax),
                        )
                o_sb = opool.tile([DV + 1, 512], f32, name="o_sb", tag="o_sb")
                nc.vector.tensor_copy(o_sb[:], po[:])
                nc.sync.dma_start(ot[h, :, csl], o_sb[:])

            # Process heads in pairs with their attention chunks interleaved:
            # while head A waits on exp (ScalarE), PE runs head B's matmuls.
            for hp in range(HPC // 2):
                hA, hB = 2 * hp, 2 * hp + 1
                qkA = qk_proj(hA)
                qkB = qk_proj(hB)
                for c in range(NTC):
                    attn_chunk(hA, *qkA, c)
                    attn_chunk(hB, *qkB, c)

    nc.finalize()
    from concourse.bass_utils import run_bass_kernel_spmd
    _cached = (nc, run_bass_kernel_spmd)
    return _cached


def _prep_core_inputs(x, Wq, Wk, Wv, core):
    b, g = core // 2, core % 2
    xb = x[b].astype(_BF16)                                  # [T, DM]
    xt = np.ascontiguousarray(
        xb.T.reshape(NDC, 128, T).transpose(1, 0, 2)         # [p, dc, t]
    )
    wq = Wq[HPC * g:HPC * (g + 1)].astype(_BF16)             # [8, DM, 64]
    wk = Wk[HPC * g:HPC * (g + 1)].astype(_BF16)
    wv = Wv[HPC * g:HPC * (g + 1)].astype(_BF16)
    wqk = np.concatenate([wq, wk], axis=2)                   # [h, DM, 128]
    wqk = np.ascontiguousarray(
        wqk.reshape(HPC, NDC, 128, 128).transpose(2, 0, 1, 3)  # [p, h, dc, f]
    )
    wvp = np.ascontiguousarray(
        wv.reshape(HPC, NDC, 128, DV).transpose(2, 1, 0, 3).reshape(128, NDC, 512)
    )
    return {"xt": xt, "wqk": wqk, "wv": wvp, "msk": _mask()}


_mask_cache = None


def _mask():
    global _mask_cache
    if _mask_cache is None:
        p = np.arange(128)[:, None, None]
        r = np.arange(4)[None, :, None]
        t = np.arange(512)[None, None, :]
        _mask_cache = (128 * r + p <= t).astype(_BF16).reshape(128, 2048)
    return _mask_cache


def kernel(x, Wq, Wk, Wv):
    global LAST_EXEC_NS
    nc, run_spmd = _build_program()
    in_maps = [_prep_core_inputs(x, Wq, Wk, Wv, c) for c in range(N_CORES)]
    res = run_spmd(nc, in_maps, list(range(N_CORES)), trace=TRACE)
    global _LAST_RES
    _LAST_RES = res
    LAST_EXEC_NS = res.exec_time_ns

    out = np.empty((B, T, H * DV), np.float32)
    for c in range(N_CORES):
        b, g = c // 2, c % 2
        otc = res.results[c]["ot"]                 # [8, 65, T]
        o = otc[:, :DV, :] / otc[:, DV:DV + 1, :]  # [h, dv, t]
        out[b, :, 512 * g:512 * (g + 1)] = (
            o.transpose(2, 0, 1).reshape(T, HPC * DV)
        )
    return out
